# revision 37
# baseline (speedup 1.0000x reference)
"""GATNet (3-layer GAT + final linear) on 8 Trainium2 NeuronCores via Bass.

Graph/data-parallel layout (per sharding hint):
  - Nodes sharded by dst across 8 cores (6250/core).  Every core keeps a full
    replica of hA_l = [h_l | alpha_src_l] (bf16, 256B-multiple rows) in DRAM;
    per-edge features are fetched with GPSIMD dma_gather (int16 indices,
    table split at row 32768 into lo/hi halves, <=1024 indices per call).
  - Per core, edges are grouped into B UNIFORM blocks of 120 dst nodes
    (fixed CH = LO+HI chunks of 128 edge slots; caps hold for the actual
    edge data and are asserted host-side).  Uniform blocks make every
    node-indexed access a compile-time slice.
  - alpha_dst is broadcast to edge slots with a second dma_gather (dst-local
    indices into a per-layer [shard+1, 128] alpha_dst table) -- no one-hot
    matrices are ever built for it.
  - Aggregation = PE matmul psum[n,:] += S_j^T @ msg_j over the block's edge
    chunks, where S (one-hot [slot x node]) is built on device from a uint8
    dstloc map and msg = [h[src]*exp(e) | exp(e)] so the same matmul yields
    the softmax denominator; normalization happens after aggregation.
  - Layer transition: x_l(shard) -> DMA-transpose -> matmul with
    Wa = [W | W@a_src | W@a_dst]; AllGather(shard) -> full hA replica.
    Layer 1 runs the same way from the per-core x shard (xsT upload).
  - Weights upload row-sharded (1/8 per core) and AllGather on device;
    biases / Wf / iota upload as single rows, log2-replicated across
    partitions with SBUF->SBUF DMAs.
  - Final linear+sigmoid fused into layer-3 epilogue; host concatenates the
    per-core [6250,1] output shards.

Per-call cost on the axon-tunneled setup is dominated by (a) input upload
at ~55-90 MB/s and (b) a ~30-65us overhead per PROGRAM instruction (code
size, not executed count -- a 2000-iteration For_i costs nothing extra), so
inputs are cut to ~0.7 MB/core and all hot loops are tc.For_i hardware
loops with ds() dynamic slices (last partial block/tile peeled): the NEFF
carries ~400 instructions instead of ~11k.  The final linear runs as its
own loop phase (x3 staged via DRAM) because the fused-epilogue variant
crashed the exec unit when placed inside a hardware loop.
"""

import numpy as np
import ml_dtypes

from concourse import bass, mybir, bacc
from concourse.bass import ds
import concourse.tile as tile
from concourse import bass_utils

BF16 = ml_dtypes.bfloat16
NEG_SLOPE = 0.2
I16_SPLIT = 32768


def rup(x, m):
    return (x + m - 1) // m * m


# ---------------------------------------------------------------- config ----


class Cfg:
    def __init__(self, N, ncores, layers, nb_n, lo_chunks, hi_chunks):
        self.N = N
        self.ncores = ncores
        self.shard = N // ncores
        assert self.shard * ncores == N
        self.layers = layers                       # [(Fin, H, C)]
        self.nb_n = nb_n                           # nodes per block (uniform)
        self.B = (self.shard + nb_n - 1) // nb_n
        self.loch, self.hich = lo_chunks, hi_chunks
        self.chunks = lo_chunks + hi_chunks
        self.lo_cap = lo_chunks * 128
        self.hi_cap = hi_chunks * 128
        assert self.lo_cap <= 1024 and self.hi_cap <= 1024
        self.losplit = I16_SPLIT if N > I16_SPLIT else N // 2
        self.colw = self.chunks * 128 // 16        # src idx16 cols per block
        self.acolw = self.chunks * 8               # dst idx16 cols per block
        self.Fs = [H * C for (_, H, C) in layers]
        self.Hs = [H for (_, H, C) in layers]
        self.rowws = [F + H for F, H in zip(self.Fs, self.Hs)]
        self.rowps = [rup(r, 128) for r in self.rowws]   # gather rows (256B)


REAL_CFG = Cfg(50000, 8, [(16, 8, 32), (256, 8, 32), (256, 12, 64)],
               120, 8, 5)


# ---------------------------------------------------------- host planning ----


def wrap16(vals, cap):
    """int16 idx stream -> wrapped [16, cap//16] layout."""
    assert len(vals) == cap and cap % 16 == 0
    return np.asarray(vals, np.int16).reshape(cap // 16, 16).T


def make_plan(cfg, edge_index):
    N, shard, CH, B = cfg.N, cfg.shard, cfg.chunks, cfg.B
    src = np.concatenate([edge_index[0].astype(np.int64), np.arange(N)])
    dst = np.concatenate([edge_index[1].astype(np.int64), np.arange(N)])
    order = np.argsort(dst, kind="stable")
    src, dst = src[order].astype(np.int64), dst[order].astype(np.int64)
    bounds = np.searchsorted(dst, np.arange(0, N + 1, shard))

    plan = {"cores": []}
    for c in range(cfg.ncores):
        e0, e1 = bounds[c], bounds[c + 1]
        csrc = src[e0:e1]
        cdstl = dst[e0:e1] - c * shard
        node_starts = np.searchsorted(cdstl, np.arange(shard + 1))

        idx16 = np.zeros((16, B * cfg.colw), np.int16)
        adidx16 = np.zeros((16, B * cfg.acolw), np.int16)
        dstloc = np.full((128, B * CH), 128, np.uint8)
        for b in range(B):
            n0, n1 = b * cfg.nb_n, min((b + 1) * cfg.nb_n, shard)
            es, ee = node_starts[n0], node_starts[n1]
            bsrc, bdstl = csrc[es:ee], cdstl[es:ee]
            blo = bsrc < cfg.losplit
            lo_src, lo_dst = bsrc[blo], bdstl[blo]
            hi_src, hi_dst = bsrc[~blo] - cfg.losplit, bdstl[~blo]
            assert len(lo_src) <= cfg.lo_cap and len(hi_src) <= cfg.hi_cap, \
                (c, b, len(lo_src), len(hi_src))
            # pad with row 0 (valid): padded slots gather bounded junk that
            # the all-zero S columns drop from the aggregation
            lo_stream = np.zeros(cfg.lo_cap, np.int64)
            lo_stream[:len(lo_src)] = lo_src
            hi_stream = np.zeros(cfg.hi_cap, np.int64)
            hi_stream[:len(hi_src)] = hi_src
            idx16[:, b * cfg.colw: b * cfg.colw + cfg.lo_cap // 16] = \
                wrap16(lo_stream, cfg.lo_cap)
            idx16[:, b * cfg.colw + cfg.lo_cap // 16:(b + 1) * cfg.colw] = \
                wrap16(hi_stream, cfg.hi_cap)
            # dst streams (slot i -> partition i%128, chunk i//128 of block)
            ad_stream = np.full(CH * 128, shard, np.int64)
            ad_stream[:len(lo_dst)] = lo_dst
            ad_stream[cfg.lo_cap:cfg.lo_cap + len(hi_dst)] = hi_dst
            adidx16[:, b * cfg.acolw:(b + 1) * cfg.acolw] = \
                wrap16(ad_stream, CH * 128)
            for sdst, base in [(lo_dst, 0), (hi_dst, cfg.lo_cap)]:
                ne = len(sdst)
                if ne == 0:
                    continue
                i = base + np.arange(ne)
                dstloc[i % 128, b * CH + i // 128] = \
                    (sdst - n0).astype(np.uint8)
        plan["cores"].append(dict(idx16=idx16, adidx16=adidx16,
                                  dstloc=dstloc))
    return plan


def fold_weights(W, a_s, a_d, H, C):
    Wr = np.asarray(W, np.float32).reshape(-1, H, C)
    ws = np.einsum("fhc,hc->fh", Wr, np.asarray(a_s, np.float32))
    wd = np.einsum("fhc,hc->fh", Wr, np.asarray(a_d, np.float32))
    return np.concatenate([Wr.reshape(Wr.shape[0], -1), ws, wd], axis=1)


def seg_split(total):
    segs, o = [], 0
    while o < total:
        w = min(512, total - o)
        segs.append((o, w))
        o += w
    return segs


# ------------------------------------------------------------ bass program ----


def build_nc(cfg):
    CH, N, shard, B = cfg.chunks, cfg.N, cfg.shard, cfg.B
    LOCH, colw, acolw = cfg.loch, cfg.colw, cfg.acolw
    dt = mybir.dt
    f32, bf16, i16, u8 = dt.float32, dt.bfloat16, dt.int16, dt.uint8

    nc = bacc.Bacc("TRN2", target_bir_lowering=False, debug=False,
                   enable_asserts=False, num_devices=cfg.ncores)

    # ---- I/O ----
    Fin1 = cfg.layers[0][0]
    FTOT = sum(cfg.Fs)
    xsT = nc.dram_tensor("xsT", [Fin1, shard], bf16, kind="ExternalInput")
    wa_rows = [cfg.layers[li][0] for li in range(3)]
    wa_cols = [cfg.Fs[li] + 2 * cfg.Hs[li] for li in range(3)]
    wa_shs = [nc.dram_tensor(f"wash{li}",
                             [wa_rows[li] // cfg.ncores, wa_cols[li]], bf16,
                             kind="ExternalInput") for li in range(3)]
    b_row = nc.dram_tensor("b_row", [1, FTOT], f32, kind="ExternalInput")
    wf_row = nc.dram_tensor("wf_row", [1, FTOT], f32, kind="ExternalInput")
    bf_row = nc.dram_tensor("bf_row", [1, 1], f32, kind="ExternalInput")
    iota_row = nc.dram_tensor("iota_row", [1, 128], bf16,
                              kind="ExternalInput")
    idx16_in = nc.dram_tensor("idx16", [16, B * colw], i16,
                              kind="ExternalInput")
    adidx16_in = nc.dram_tensor("adidx16", [16, B * acolw], i16,
                                kind="ExternalInput")
    dstloc_in = nc.dram_tensor("dstloc", [128, B * CH], u8,
                               kind="ExternalInput")
    out = nc.dram_tensor("out", [shard, 1], f32, kind="ExternalOutput")

    # ---- internal DRAM ----
    wa_shi = [nc.dram_tensor(f"washi{li}",
                             [wa_rows[li] // cfg.ncores, wa_cols[li]],
                             bf16, kind="Internal") for li in range(3)]
    wa_full = [nc.dram_tensor(f"waf{li}", [wa_rows[li], wa_cols[li]], bf16,
                              kind="Internal", addr_space="Shared")
               for li in range(3)]
    hA_full = [nc.dram_tensor(f"hAfull{li}", [N, cfg.rowps[li]], bf16,
                              kind="Internal", addr_space="Shared")
               for li in range(3)]
    hA_shard = [nc.dram_tensor(f"hAshard{li}", [shard, cfg.rowps[li]],
                               bf16, kind="Internal") for li in range(3)]
    # node-major x tables (x_1, x_2) and per-layer alpha_dst gather tables
    xtab = [None] + [nc.dram_tensor(f"xtab{li}", [shard + 128, 256],
                                    bf16, kind="Internal") for li in (1, 2)]
    adt = [nc.dram_tensor(f"adt{li}", [shard + 1, 128], bf16,
                          kind="Internal") for li in range(3)]
    xtab3 = nc.dram_tensor("xtab3", [shard + 128, cfg.Fs[2]], bf16,
                           kind="Internal")

    with tile.TileContext(nc) as tc:
        with tc.tile_pool(name="const", bufs=1) as cpool, \
             tc.tile_pool(name="io", bufs=3) as iop, \
             tc.tile_pool(name="gath", bufs=2) as gp, \
             tc.tile_pool(name="fetch", bufs=2) as fp, \
             tc.tile_pool(name="work", bufs=2) as wp, \
             tc.tile_pool(name="small", bufs=3) as sp, \
             tc.tile_pool(name="psum", bufs=2, space="PSUM") as pp:

            # ---- weights: AllGather the row-shards ----
            for li in range(3):
                nc.sync.dma_start(out=wa_shi[li][:, :], in_=wa_shs[li][:, :])
                nc.gpsimd.collective_compute(
                    "AllGather", mybir.AluOpType.bypass,
                    replica_groups=[list(range(cfg.ncores))],
                    ins=[wa_shi[li][:]], outs=[wa_full[li][:]])

            # ---- small replicated constants: row upload + log2 replicate --
            def replicate_rows(t, width):
                p = 1
                while p < 128:
                    nc.sync.dma_start(out=t[p:2 * p, :width],
                                      in_=t[0:p, :width])
                    p *= 2

            # pack [bias | wf] into one f32 tile, one replicate chain
            pack_sb = cpool.tile([128, 2 * FTOT + 1], f32)
            nc.sync.dma_start(out=pack_sb[0:1, :FTOT], in_=b_row[:, :])
            nc.sync.dma_start(out=pack_sb[0:1, FTOT:2 * FTOT],
                              in_=wf_row[:, :])
            nc.sync.dma_start(out=pack_sb[0:1, 2 * FTOT:], in_=bf_row[:, :])
            replicate_rows(pack_sb, 2 * FTOT + 1)
            bias_sb = pack_sb[:, :FTOT]
            wf_sb = pack_sb[:, FTOT:2 * FTOT]
            bf_sb = pack_sb[:, 2 * FTOT:]
            iota_sb = cpool.tile([128, 128], bf16)
            nc.sync.dma_start(out=iota_sb[0:1, :], in_=iota_row[:, :])
            replicate_rows(iota_sb, 128)
            boffs = [0, cfg.Fs[0], cfg.Fs[0] + cfg.Fs[1]]

            # ---- idx streams: replicate compact [16, .] to 128 partitions -
            idx_sb = cpool.tile([128, B * colw], i16)
            adidx_sb = cpool.tile([128, B * acolw], i16)
            for k in range(8):
                nc.sync.dma_start(out=idx_sb[16 * k:16 * (k + 1), :],
                                  in_=idx16_in[:, :])
                nc.sync.dma_start(out=adidx_sb[16 * k:16 * (k + 1), :],
                                  in_=adidx16_in[:, :])

            # ---- dstloc: u8 upload -> bf16 ----
            dstloc8_sb = cpool.tile([128, B * CH], u8)
            nc.sync.dma_start(out=dstloc8_sb[:], in_=dstloc_in[:, :])
            dstloc_sb = cpool.tile([128, B * CH], bf16)
            nc.vector.tensor_copy(out=dstloc_sb[:], in_=dstloc8_sb[:])

            # ---- zero x-table tails and adt padding rows ----
            ztail = cpool.tile([128, 256], bf16)
            nc.vector.memset(ztail[:], 0.0)
            for li in (1, 2):
                nc.sync.dma_start(out=xtab[li][shard:shard + 128, :],
                                  in_=ztail[:, :])
            for li in range(3):
                nc.sync.dma_start(out=adt[li][shard:shard + 1, :],
                                  in_=ztail[:1, :128])

            # ---- weight tiles to SBUF (after AllGather) -------------------
            wa_sb = []
            for li in range(3):
                Fin = cfg.layers[li][0]
                tiles = []
                for f0 in range(0, Fin, 128):
                    w = min(128, Fin - f0)
                    t = cpool.tile([128, wa_cols[li]], bf16,
                                   tag=f"wa{li}_{f0}", name=f"wa{li}_{f0}")
                    nc.sync.dma_start(out=t[:w], in_=wa_full[li][f0:f0 + w, :])
                    tiles.append((t, w))
                wa_sb.append(tiles)

            # ---------------- phase B1: hA1 = x_shard @ Wa1 + AllGather ----
            roww0 = cfg.rowws[0]
            wa1_t = wa_sb[0][0][0]
            H1 = cfg.Hs[0]
            t_full = shard // 128 * 128          # full 128-row tiles

            def b1_body(t0, w):
                lhs = iop.tile([Fin1, 128], bf16, tag="b1lhs")
                if w < 128:
                    nc.vector.memset(lhs[:], 0.0)
                nc.sync.dma_start(out=lhs[:, :w], in_=xsT[:, ds(t0, w)])
                ph = pp.tile([128, cfg.Fs[0] + 2 * H1], f32, tag="agg0")
                nc.tensor.matmul(out=ph[:], lhsT=lhs[:], rhs=wa1_t[:Fin1],
                                 start=True, stop=True)
                hcp = iop.tile([128, roww0], bf16, tag="b1h")
                nc.vector.tensor_copy(out=hcp[:w], in_=ph[:w, :roww0])
                nc.sync.dma_start(out=hA_shard[0][ds(t0, w), :roww0],
                                  in_=hcp[:w])
                acp = sp.tile([128, H1], bf16, tag="bacp")
                nc.vector.tensor_copy(out=acp[:w],
                                      in_=ph[:w, roww0:roww0 + H1])
                nc.sync.dma_start(out=adt[0][ds(t0, w), 0:H1], in_=acp[:w])

            with tc.For_i(0, t_full, 128) as t0v:
                b1_body(t0v, 128)
            if t_full < shard:
                b1_body(t_full, shard - t_full)
            nc.gpsimd.collective_compute(
                "AllGather", mybir.AluOpType.bypass,
                replica_groups=[list(range(cfg.ncores))],
                ins=[hA_shard[0][:]], outs=[hA_full[0][:]])

            # ---------------- layers ----------------------------------------
            for li in range(3):
                Fin, H, C = cfg.layers[li]
                F, rowp = cfg.Fs[li], cfg.rowps[li]
                segs = seg_split(F + H)
                is_last = li == 2
                nbn = cfg.nb_n
                adspl = (CH // 2) * 128         # dst gather split (<=1024)

                def block_body(b, n0, nn, li=li, F=F, rowp=rowp, segs=segs,
                               is_last=is_last, H=H, C=C, adspl=adspl):
                    hg = gp.tile([128, CH, rowp], bf16, tag="hg")
                    nc.gpsimd.dma_gather(
                        hg[:, :LOCH, :], hA_full[li][0:cfg.losplit, :],
                        idx_sb[:, ds(b * colw, cfg.lo_cap // 16)],
                        cfg.lo_cap, cfg.lo_cap, rowp)
                    nc.gpsimd.dma_gather(
                        hg[:, LOCH:, :], hA_full[li][cfg.losplit:N, :],
                        idx_sb[:, ds(b * colw + cfg.lo_cap // 16,
                                     cfg.hi_cap // 16)],
                        cfg.hi_cap, cfg.hi_cap, rowp)

                    # alpha_dst per edge slot: gather from adt by dst idx
                    adg = fp.tile([128, CH, 128], bf16, tag="adg")
                    nc.gpsimd.dma_gather(
                        adg[:, :CH // 2, :], adt[li][:, :],
                        adidx_sb[:, ds(b * acolw, adspl // 16)],
                        adspl, adspl, 128)
                    nc.gpsimd.dma_gather(
                        adg[:, CH // 2:, :], adt[li][:, :],
                        adidx_sb[:, ds(b * acolw + adspl // 16,
                                       (CH * 128 - adspl) // 16)],
                        CH * 128 - adspl, CH * 128 - adspl, 128)

                    # one-hot S for aggregation
                    S_sb = wp.tile([128, CH * 128], bf16, tag="S")
                    iota_b = bass.AP(iota_sb[:].tensor, iota_sb[:].offset,
                                     [iota_sb[:].ap[0], [0, CH], [1, 128]])
                    nc.vector.tensor_tensor(
                        out=S_sb[:].rearrange("p (c n) -> p c n", c=CH),
                        in0=dstloc_sb[:, ds(b * CH, CH)]
                        .to_broadcast([128, CH, 128]),
                        in1=iota_b, op=mybir.AluOpType.is_equal)

                    # e = lrelu(alpha_s[src] + alpha_d[dst]); ex = exp(e)
                    et = sp.tile([128, CH * H], f32, tag="et")
                    nc.vector.tensor_tensor(
                        out=et[:].rearrange("p (c h) -> p c h", c=CH),
                        in0=hg[:, :, F:F + H],
                        in1=adg[:, :, :H],
                        op=mybir.AluOpType.add)
                    nc.vector.scalar_tensor_tensor(
                        out=et[:], in0=et[:], scalar=NEG_SLOPE, in1=et[:],
                        op0=mybir.AluOpType.mult, op1=mybir.AluOpType.max)
                    ex = sp.tile([128, CH * H], bf16, tag="ex")
                    nc.scalar.activation(
                        out=ex[:], in_=et[:],
                        func=mybir.ActivationFunctionType.Exp)

                    # msg = [hg * ex | ex]
                    msg = gp.tile([128, CH, F + H], bf16, tag="msg")
                    ex3 = ex[:].rearrange("p (c h) -> p c h", c=CH)
                    nc.vector.tensor_tensor(
                        out=msg[:, :, :F].rearrange(
                            "p c (h k) -> p c h k", h=H),
                        in0=hg[:, :, :F].rearrange(
                            "p c (h k) -> p c h k", h=H),
                        in1=ex3.to_broadcast([128, CH, H, C]),
                        op=mybir.AluOpType.mult)
                    nc.vector.tensor_copy(out=msg[:, :, F:], in_=ex3)

                    # aggregate
                    pts = [pp.tile([128, w], f32, tag=f"agg{si}",
                                   name=f"agg{si}")
                           for si, (o, w) in enumerate(segs)]
                    for j in range(CH):
                        lhsT = S_sb[:, j * 128:(j + 1) * 128]
                        for (o, w), pt in zip(segs, pts):
                            nc.tensor.matmul(out=pt[:], lhsT=lhsT,
                                             rhs=msg[:, j, o:o + w],
                                             start=(j == 0),
                                             stop=(j == CH - 1))

                    # normalize + bias + relu  (no EPS: self-loops => den>0)
                    dseg = len(segs) - 1
                    dof = F - segs[dseg][0]
                    rec = sp.tile([128, H], f32, tag="rec")
                    nc.vector.reciprocal(out=rec[:],
                                         in_=pts[dseg][:, dof:dof + H])
                    xt = wp.tile([128, F], f32, tag="xt")
                    for si, (o, w) in enumerate(segs):
                        wF = min(w, F - o)
                        h0, nh = o // C, min(w, F - o) // C
                        nc.vector.tensor_tensor(
                            out=xt[:, o:o + wF].rearrange(
                                "p (h k) -> p h k", h=nh),
                            in0=pts[si][:, :wF].rearrange(
                                "p (h k) -> p h k", h=nh),
                            in1=rec[:, h0:h0 + nh].to_broadcast(
                                [128, nh, C]),
                            op=mybir.AluOpType.mult)
                    nc.vector.tensor_tensor(
                        out=xt[:], in0=xt[:],
                        in1=bias_sb[:, boffs[li]:boffs[li] + F],
                        op=mybir.AluOpType.add)
                    x_bf = wp.tile([128, F], bf16, tag="xbf")
                    nc.vector.tensor_scalar_max(out=x_bf[:], in0=xt[:],
                                                scalar1=0.0)

                    xdst = xtab[li + 1] if not is_last else xtab3
                    nc.sync.dma_start(out=xdst[ds(n0, nn), :],
                                      in_=x_bf[:nn])

                with tc.For_i(0, B - 1) as bv:
                    block_body(bv, bv * nbn, nbn)
                block_body(B - 1, (B - 1) * nbn, shard - (B - 1) * nbn)

                # ------- phase B(l+1) + AllGather --------------------------
                if not is_last:
                    lin = li + 1
                    Fn, Hn = cfg.Fs[lin], cfg.Hs[lin]
                    rowwn = cfg.rowws[lin]
                    nsegs = seg_split(Fn + 2 * Hn)

                    def trans_body(t0, w, li=li, lin=lin, F=F, Fn=Fn, Hn=Hn,
                                   rowwn=rowwn, nsegs=nsegs):
                        phs = [pp.tile([128, w2], f32, tag=f"agg{si}",
                                       name=f"bh{si}")
                               for si, (o2, w2) in enumerate(nsegs)]
                        for fi in range(F // 128):
                            xTs = iop.tile([128, 128], bf16, tag="bxT")
                            # full 128 rows: the zeroed tail keeps the xbar
                            # transpose 16-row-aligned on the last tile
                            nc.sync.dma_start_transpose(
                                out=xTs[:],
                                in_=xtab[li + 1][ds(t0, 128),
                                                 fi * 128:(fi + 1) * 128])
                            wa_t, ww = wa_sb[lin][fi]
                            for (o2, w2), ph2 in zip(nsegs, phs):
                                nc.tensor.matmul(
                                    out=ph2[:], lhsT=xTs[:],
                                    rhs=wa_t[:ww, o2:o2 + w2],
                                    start=(fi == 0),
                                    stop=(fi == F // 128 - 1))
                        hcp = iop.tile([128, rowwn], bf16, tag="bhcp")
                        for si, (o2, w2) in enumerate(nsegs):
                            wh = min(w2, rowwn - o2)
                            if wh > 0:
                                nc.vector.tensor_copy(
                                    out=hcp[:w, o2:o2 + wh],
                                    in_=phs[si][:w, :wh])
                        acp = sp.tile([128, Hn], bf16, tag="bacp")
                        dseg2 = len(nsegs) - 1
                        dof2 = rowwn - nsegs[dseg2][0]
                        nc.vector.tensor_copy(
                            out=acp[:w], in_=phs[dseg2][:w, dof2:dof2 + Hn])
                        nc.sync.dma_start(
                            out=hA_shard[lin][ds(t0, w), :rowwn],
                            in_=hcp[:w])
                        nc.sync.dma_start(
                            out=adt[lin][ds(t0, w), 0:Hn], in_=acp[:w])

                    with tc.For_i(0, t_full, 128) as t0v:
                        trans_body(t0v, 128)
                    if t_full < shard:
                        trans_body(t_full, shard - t_full)
                    nc.gpsimd.collective_compute(
                        "AllGather", mybir.AluOpType.bypass,
                        replica_groups=[list(range(cfg.ncores))],
                        ins=[hA_shard[lin][:]], outs=[hA_full[lin][:]])

            # ---------------- final linear + sigmoid -----------------------
            F01, F02 = cfg.Fs[0], cfg.Fs[0] + cfg.Fs[1]

            def final_body(t0, w):
                x1t = fp.tile([128, cfg.Fs[0]], bf16, tag="x1sb")
                nc.sync.dma_start(out=x1t[:], in_=xtab[1][ds(t0, 128), :])
                x2t = fp.tile([128, cfg.Fs[1]], bf16, tag="x2sb")
                nc.sync.dma_start(out=x2t[:], in_=xtab[2][ds(t0, 128), :])
                x3t = fp.tile([128, cfg.Fs[2]], bf16, tag="x3sb")
                nc.sync.dma_start(out=x3t[:], in_=xtab3[ds(t0, 128), :])
                scratch = wp.tile([128, FTOT], bf16, tag="fscratch",
                                  bufs=1)
                nc.vector.tensor_tensor(
                    out=scratch[:, :F01], in0=x1t[:],
                    in1=wf_sb[:, :F01], op=mybir.AluOpType.mult)
                nc.vector.tensor_tensor(
                    out=scratch[:, F01:F02], in0=x2t[:],
                    in1=wf_sb[:, F01:F02], op=mybir.AluOpType.mult)
                nc.vector.tensor_tensor(
                    out=scratch[:, F02:], in0=x3t[:],
                    in1=wf_sb[:, F02:], op=mybir.AluOpType.mult)
                acc = sp.tile([128, 1], f32, tag="acc")
                nc.vector.reduce_sum(out=acc[:], in_=scratch[:],
                                     axis=mybir.AxisListType.X)
                res = sp.tile([128, 1], f32, tag="res")
                nc.scalar.activation(
                    out=res[:], in_=acc[:],
                    func=mybir.ActivationFunctionType.Sigmoid,
                    bias=bf_sb[:, 0:1])
                nc.sync.dma_start(out=out[ds(t0, w), :], in_=res[:w])

            with tc.For_i(0, t_full, 128) as t0v:
                final_body(t0v, 128)
            if t_full < shard:
                final_body(t_full, shard - t_full)

    nc.compile()
    return nc


# ------------------------------------------------------------- host entry ----


def make_inputs(cfg, plan, x, W1, as1, ad1, b1, W2, as2, ad2, b2,
                W3, as3, ad3, b3, Wf, bf):
    x = np.asarray(x, np.float32)
    Wa = [fold_weights(W, a_s, a_d, H, C).astype(BF16)
          for (W, a_s, a_d, (Fin, H, C)) in
          [(W1, as1, ad1, cfg.layers[0]), (W2, as2, ad2, cfg.layers[1]),
           (W3, as3, ad3, cfg.layers[2])]]
    FTOT = sum(cfg.Fs)
    b_row = np.concatenate([np.asarray(b, np.float32).ravel()
                            for b in (b1, b2, b3)]).reshape(1, FTOT)
    wf_row = np.asarray(Wf, np.float32).reshape(1, FTOT)
    bf_row = np.asarray(bf, np.float32).reshape(1, 1)
    iota_row = np.arange(128, dtype=np.float32).astype(BF16).reshape(1, 128)
    shard = cfg.shard
    in_maps = []
    for c in range(cfg.ncores):
        pc = plan["cores"][c]
        xs = x[c * shard:(c + 1) * shard].astype(BF16)
        m = dict(xsT=np.ascontiguousarray(xs.T),
                 b_row=b_row, wf_row=wf_row, bf_row=bf_row,
                 iota_row=iota_row,
                 idx16=pc["idx16"], adidx16=pc["adidx16"],
                 dstloc=pc["dstloc"])
        for li in range(3):
            rows = Wa[li].shape[0] // cfg.ncores
            m[f"wash{li}"] = np.ascontiguousarray(
                Wa[li][c * rows:(c + 1) * rows])
        in_maps.append(m)
    return in_maps


_CACHE = {}


def _get_compiled(cfg, edge_index):
    key = hash(np.asarray(edge_index).tobytes())
    if key not in _CACHE:
        plan = make_plan(cfg, np.asarray(edge_index))
        nc = build_nc(cfg)
        _CACHE.clear()
        _CACHE[key] = (plan, nc)
    return _CACHE[key]


def kernel(x, edge_index, JetRawPt, W1, as1, ad1, b1, W2, as2, ad2, b2,
           W3, as3, ad3, b3, Wf, bf):
    cfg = REAL_CFG
    plan, nc = _get_compiled(cfg, np.asarray(edge_index))
    in_maps = make_inputs(cfg, plan, x, W1, as1, ad1, b1, W2, as2, ad2, b2,
                          W3, as3, ad3, b3, Wf, bf)
    res = bass_utils.run_bass_kernel_spmd(nc, in_maps,
                                          core_ids=list(range(cfg.ncores)))
    return np.concatenate([res.results[c]["out"]
                           for c in range(cfg.ncores)], axis=0)
